# revision 2
# baseline (speedup 1.0000x reference)
"""GCN layer (GCNConv + BatchNorm + ReLU + residual) as a multi-core TRN2 Bass kernel.

v2 design — reformulated to aggregate-then-transform:
    h = D^-1/2 (A+I) D^-1/2 (x W) = (D^-1/2 (A+I) D^-1/2 x) W
so the gather table z = dinv * x is PURE INPUT DATA (host-precomputed,
replicated to all cores) and no AllGather is needed.  The only collective is
a 1KB AllReduce of BN statistics.

Aggregation is round-based with three compression tricks:
  * pair rows: the z table is stored as [25001, 256] bf16 (two nodes per
    512B row, full DMA line rate; pair index fits int16).  Rounds are
    parity-designated: an even round only gathers even-id srcs (half 0 of
    the fetched pair), an odd round odd srcs, so the wanted half is uniform
    per round and selected by the access pattern - no masks.
  * degree-sorted prefixes: each core relabels its nodes by descending
    max(even-indeg, odd-indeg), so round t only covers the slot prefix that
    still has a t-th in-edge.  Gathered slots ~= E + small padding instead
    of R * N.
  * grouped calls: (round, parity) segments are packed into transpose-mode
    dma_gather calls of up to 6272 indices; each segment's wanted half is one
    strided-slice DVE add into the accumulator.

Everything on chip is feature-major ([128 features x slots]), so:
  * xW is 13 matmuls with the small FIXED W as the stationary operand,
  * dinv_dst is applied during the PSUM drain (dinv commutes past W),
  * BN stats are two free-axis reduces + AllReduce([128,2]),
  * BN apply + ReLU is one scalar-engine activation with per-partition
    scale/bias; the residual is one add with the raw xT input.

Host does layout only: permutations, padding, index/table construction, and
un-permuting the output.  b cancels under training-mode BN and is skipped.
"""

import numpy as np
import ml_dtypes

import concourse.bass as bass
import concourse.bacc as bacc
import concourse.mybir as mybir
import concourse.tile as tile

P = 128
BN_EPS = 1e-5


def cdiv(a, b):
    return -(-a // b)


class Plan:
    pass


# ---------------------------------------------------------------------------
# Host-side preprocessing: pure index/layout manipulation.
# ---------------------------------------------------------------------------

def preprocess(x, W, gamma, beta, edge_index, n_cores=8, group_pairs=2):
    x = np.ascontiguousarray(np.asarray(x), dtype=np.float32)
    W = np.ascontiguousarray(np.asarray(W), dtype=np.float32)
    gamma = np.asarray(gamma, dtype=np.float32).reshape(-1)
    beta = np.asarray(beta, dtype=np.float32).reshape(-1)
    ei = np.asarray(edge_index)
    src = ei[0].astype(np.int64)
    dst = ei[1].astype(np.int64)

    N, D = x.shape
    assert D == P and N % (2 * n_cores) == 0
    SHARD = N // n_cores
    NW = cdiv(SHARD, P)
    PADN = NW * P

    deg = (np.bincount(dst, minlength=N) + 1).astype(np.float32)
    dinv = 1.0 / np.sqrt(deg)
    z = x * dinv[:, None]

    NPAIR = N // 2          # data pairs
    ZPAIR = NPAIR           # zero row index
    table = np.zeros((NPAIR + 1, 2 * D), np.float32)
    table[:NPAIR] = z.reshape(NPAIR, 2 * D)
    table = table.astype(ml_dtypes.bfloat16)

    # ---- per-core edge lists split by src parity, degree-sorted slots ----
    core_of = dst // SHARD
    dloc = (dst - core_of * SHARD).astype(np.int64)
    par = (src % 2).astype(np.int64)

    # per (core, local node): in-edge srcs by parity.  Use stable sort by
    # (core, dloc) and slice.
    edge_order = np.lexsort((par, dloc, core_of))
    so, co, dl, pa = (src[edge_order], core_of[edge_order],
                      dloc[edge_order], par[edge_order])

    d0 = np.zeros((n_cores, SHARD), np.int64)
    d1 = np.zeros((n_cores, SHARD), np.int64)
    np.add.at(d0, (co[pa == 0], dl[pa == 0]), 1)
    np.add.at(d1, (co[pa == 1], dl[pa == 1]), 1)
    dmax = np.maximum(d0, d1)
    R_pairs = int(dmax.max())

    # slot permutation per core: by descending max(d0, d1)
    perm = np.argsort(-dmax, axis=1, kind="stable")       # [cores, SHARD]

    # per-(core,node,parity,t) src matrix, in slot order: [cores, SHARD, 2, R_pairs]
    srcs = np.full((n_cores, SHARD, 2, R_pairs), -1, np.int64)
    # position of each edge within its (core,node,parity) bucket:
    key = ((co * SHARD + dl) * 2 + pa)
    start = np.searchsorted(key, key)      # first occurrence index (sorted)
    t_of = np.arange(len(key)) - start
    srcs[co, dl, pa, t_of] = so
    srcs = srcs[np.arange(n_cores)[:, None], perm]        # slot order

    d0s = d0[np.arange(n_cores)[:, None], perm]
    d1s = d1[np.arange(n_cores)[:, None], perm]

    # prefixes per (parity, round-pair): slots needed, rounded to 128, and
    # shared across cores (single SPMD program).
    pref = np.zeros((2, R_pairs), np.int64)
    for t in range(R_pairs):
        m0 = d0s > t
        m1 = d1s > t
        k0 = int(max((np.nonzero(m)[0][-1] + 1 if m.any() else 0)
                     for m in m0))
        k1 = int(max((np.nonzero(m)[0][-1] + 1 if m.any() else 0)
                     for m in m1))
        pref[0, t] = cdiv(max(k0, 1), P) * P
        pref[1, t] = cdiv(max(k1, 1), P) * P

    # pack (round-pair, parity) segments into gather calls of <= CAP idxs
    CAP = 6272
    segs = [(t, q, int(pref[q, t])) for t in range(R_pairs) for q in (0, 1)]
    calls = []    # (S, ((t, q), ...))
    cur, curS = [], 0
    for (t, q, s) in segs:
        S_new = max(curS, s)
        if cur and (len(cur) + 1) * S_new > CAP:
            calls.append((curS, tuple(cur)))
            cur, curS = [], 0
            S_new = s
        cur.append((t, q))
        curS = S_new
    if cur:
        calls.append((curS, tuple(cur)))
    total_idx = sum(S * len(sg) for S, sg in calls)

    # ---- per-core idx arrays + per-core tensors ----
    plan = Plan()
    plan.n_cores, plan.N, plan.D = n_cores, N, D
    plan.SHARD, plan.NW, plan.PADN = SHARD, NW, PADN
    plan.NPAIR_ROWS = NPAIR + 1
    plan.R_pairs, plan.calls = R_pairs, calls
    plan.idx_cols = total_idx // 16
    plan.perm = perm

    in_maps = []
    for c in range(n_cores):
        idx16 = np.full((16, plan.idx_cols), ZPAIR, np.int16)
        col = 0
        for (S, segsg) in calls:
            parts = []
            for (t, q) in segsg:
                seg = np.full(S, 2 * ZPAIR, np.int64)
                valid = min(S, SHARD)
                sv = srcs[c, :valid, q, t]
                m = sv >= 0
                seg[:valid][m] = sv[m]
                parts.append(seg)
            gi = np.concatenate(parts) // 2    # pair indices
            nidx = len(gi)
            blk = gi.reshape(nidx // 16, 16).T.astype(np.int16)
            idx16[:, col:col + nidx // 16] = blk
            col += nidx // 16
        assert col == plan.idx_cols
        idxT = np.tile(idx16, (8, 1))

        pc = perm[c]
        nodes = pc + c * SHARD
        zx = np.zeros((P, 2 * PADN), np.float32)
        zx[:, :SHARD] = z[nodes].T                 # zT (acc init)
        zx[:, PADN:PADN + SHARD] = x[nodes].T      # xT (residual)
        dbc = np.zeros((P, PADN), np.float32)
        dbc[:, :SHARD] = dinv[nodes][None, :]
        dbc = dbc.astype(ml_dtypes.bfloat16)
        wgb = np.zeros((P, D + 2), np.float32)
        wgb[:, :D] = W
        wgb[:, D] = gamma
        wgb[:, D + 1] = beta

        in_maps.append({
            "ztab": table,
            "zx": zx,
            "dbc": dbc,
            "wgb": wgb,
            "idxT": idxT,
        })
    return plan, in_maps


# ---------------------------------------------------------------------------
# Bass program (SPMD, one program for all cores)
# ---------------------------------------------------------------------------

def build_nc(plan, reps=1, no_coll=False, no_gather=False):
    dt = mybir.dt
    f32, b16, i16 = dt.float32, dt.bfloat16, dt.int16
    NW, PADN, SHARD, D, N = plan.NW, plan.PADN, plan.SHARD, plan.D, plan.N
    NCHUNK = cdiv(PADN, 512)
    MAXNIDX = max(S * len(sg) for S, sg in plan.calls)
    rg = [list(range(plan.n_cores))]

    nc = bacc.Bacc("TRN2", target_bir_lowering=False, debug=False,
                   num_devices=plan.n_cores)

    ztab = nc.dram_tensor("ztab", [plan.NPAIR_ROWS, 2 * D], b16,
                          kind="ExternalInput")
    zx = nc.dram_tensor("zx", [P, 2 * PADN], f32, kind="ExternalInput")
    dbc = nc.dram_tensor("dbc", [P, PADN], b16, kind="ExternalInput")
    wgb = nc.dram_tensor("wgb", [P, D + 2], f32, kind="ExternalInput")
    idxT = nc.dram_tensor("idxT", [P, plan.idx_cols], i16, kind="ExternalInput")
    outT = nc.dram_tensor("outT", [P, PADN], f32, kind="ExternalOutput")

    with tile.TileContext(nc) as tc:
        with (
            tc.tile_pool(name="const", bufs=1) as cpool,
            tc.tile_pool(name="acc", bufs=1) as apool,
            tc.tile_pool(name="big", bufs=1) as big,
            tc.tile_pool(name="dram", bufs=1, space="DRAM") as dram,
            tc.tile_pool(name="gbuf", bufs=2) as gpool,
            tc.tile_pool(name="ps", bufs=4, space="PSUM") as pspool,
        ):
            wgb_sb = cpool.tile([P, D + 2], f32)
            dinv_sb = cpool.tile([P, PADN], b16)
            idx_sb = cpool.tile([P, plan.idx_cols], i16)
            s12_sb = cpool.tile([P, 2], f32)
            ar_sb = cpool.tile([P, 2], f32)
            mean_sb = cpool.tile([P, 1], f32)
            var_sb = cpool.tile([P, 1], f32)
            istd_sb = cpool.tile([P, 1], f32)
            scale_sb = cpool.tile([P, 1], f32)
            shift_sb = cpool.tile([P, 1], f32)
            zero_sb = cpool.tile([P, 1], f32)

            h_sb = big.tile([P, PADN], f32)

            for _rep in range(reps):
                stats_in = dram.tile([P, 2], f32, tag="sti", name="sti")
                stats_out = dram.tile([P, 2], f32, addr_space="Shared",
                                      tag="sto", name="sto")

                # ---- input loads ----
                acc = apool.tile([P, PADN], f32, tag="A", name="acc")
                nc.vector.memset(zero_sb[:], 0.0)
                nc.sync.dma_start(out=wgb_sb[:], in_=wgb.ap())
                nc.sync.dma_start(out=dinv_sb[:], in_=dbc.ap())
                nc.sync.dma_start(out=idx_sb[:], in_=idxT.ap())
                nc.sync.dma_start(out=acc[:], in_=zx.ap()[:, 0:PADN])

                # ---- gather rounds: acc += sum_r z[src_r] ----
                zview = ztab.ap()[0:plan.NPAIR_ROWS, :]
                c0 = 0
                for (S, segsg) in plan.calls:
                    nidx = S * len(segsg)
                    if no_gather:
                        c0 += nidx // 16
                        continue
                    buf = gpool.tile([P, 2 * MAXNIDX], b16, tag="gb",
                                     name="gb")
                    bview = bass.AP(buf[:].tensor, buf[:].offset,
                                    [buf[:].ap[0], [nidx, 2], [1, nidx]])
                    nc.gpsimd.dma_gather(
                        bview, zview, idx_sb[:, c0:c0 + nidx // 16],
                        num_idxs=nidx, num_idxs_reg=nidx,
                        elem_size=2 * D, transpose=True, single_packet=False)
                    c0 += nidx // 16
                    # segment j's wanted half q_j at free offset
                    # q_j*nidx + j*S, length S
                    for j, (t, q) in enumerate(segsg):
                        seg = bass.AP(buf[:].tensor,
                                      buf[:].offset + q * nidx + j * S,
                                      [buf[:].ap[0], [1, S]])
                        nc.vector.tensor_tensor(
                            out=acc[:, 0:S], in0=acc[:, 0:S], in1=seg,
                            op=mybir.AluOpType.add)

                # ---- h = W^T acc, scaled by dinv_dst during PSUM drain ----
                for k in range(NCHUNK):
                    c0, c1 = k * 512, min(PADN, (k + 1) * 512)
                    ps = pspool.tile([P, 512], f32, tag="ps")
                    nc.tensor.matmul(ps[:, 0:c1 - c0], lhsT=wgb_sb[:, 0:D],
                                     rhs=acc[:, c0:c1], start=True, stop=True)
                    nc.vector.tensor_tensor(out=h_sb[:, c0:c1],
                                            in0=ps[:, 0:c1 - c0],
                                            in1=dinv_sb[:, c0:c1],
                                            op=mybir.AluOpType.mult)

                # ---- BN stats + AllReduce ----
                sq = apool.tile([P, PADN], f32, tag="A", name="sq")
                nc.scalar.activation(out=sq[:], in_=h_sb[:],
                                     func=mybir.ActivationFunctionType.Identity,
                                     bias=zero_sb[:, 0:1],
                                     accum_out=s12_sb[:, 0:1])
                nc.scalar.activation(out=sq[:], in_=h_sb[:],
                                     func=mybir.ActivationFunctionType.Square,
                                     bias=zero_sb[:, 0:1],
                                     accum_out=s12_sb[:, 1:2])
                nc.sync.dma_start(out=stats_in[0:P, :], in_=s12_sb[:])
                if no_coll:
                    nc.sync.dma_start(out=stats_out[0:P, :],
                                      in_=stats_in[0:P, :])
                else:
                    nc.gpsimd.collective_compute(
                        "AllReduce", mybir.AluOpType.add, replica_groups=rg,
                        ins=[stats_in.opt()], outs=[stats_out.opt()])
                nc.sync.dma_start(out=ar_sb[:], in_=stats_out[0:P, :])

                # ---- BN scalars ----
                inv_n = 1.0 / float(N)
                nc.vector.tensor_scalar(out=mean_sb[:], in0=ar_sb[:, 0:1],
                                        scalar1=inv_n, scalar2=None,
                                        op0=mybir.AluOpType.mult)
                nc.vector.tensor_scalar(out=var_sb[:], in0=ar_sb[:, 1:2],
                                        scalar1=inv_n, scalar2=None,
                                        op0=mybir.AluOpType.mult)
                nc.vector.tensor_tensor(out=istd_sb[:], in0=mean_sb[:],
                                        in1=mean_sb[:],
                                        op=mybir.AluOpType.mult)
                nc.vector.tensor_tensor(out=var_sb[:], in0=var_sb[:],
                                        in1=istd_sb[:],
                                        op=mybir.AluOpType.subtract)
                nc.vector.tensor_scalar(out=var_sb[:], in0=var_sb[:],
                                        scalar1=float(BN_EPS), scalar2=None,
                                        op0=mybir.AluOpType.add)
                nc.scalar.activation(out=istd_sb[:], in_=var_sb[:],
                                     func=mybir.ActivationFunctionType.Sqrt,
                                     bias=zero_sb[:, 0:1])
                nc.vector.reciprocal(out=istd_sb[:], in_=istd_sb[:])
                nc.vector.tensor_tensor(out=scale_sb[:],
                                        in0=wgb_sb[:, D:D + 1],
                                        in1=istd_sb[:],
                                        op=mybir.AluOpType.mult)
                nc.vector.tensor_tensor(out=shift_sb[:], in0=mean_sb[:],
                                        in1=scale_sb[:],
                                        op=mybir.AluOpType.mult)
                nc.vector.tensor_tensor(out=shift_sb[:],
                                        in0=wgb_sb[:, D + 1:D + 2],
                                        in1=shift_sb[:],
                                        op=mybir.AluOpType.subtract)

                # ---- out = relu(h*scale + shift) + x ----
                nc.scalar.activation(out=h_sb[:], in_=h_sb[:],
                                     func=mybir.ActivationFunctionType.Relu,
                                     bias=shift_sb[:, 0:1],
                                     scale=scale_sb[:, 0:1])
                xres = apool.tile([P, PADN], f32, tag="A", name="xres")
                nc.sync.dma_start(out=xres[:], in_=zx.ap()[:, PADN:2 * PADN])
                nc.vector.tensor_tensor(out=h_sb[:], in0=h_sb[:],
                                        in1=xres[:], op=mybir.AluOpType.add)
                nc.sync.dma_start(out=outT.ap(), in_=h_sb[:])

    nc.compile()
    return nc


# ---------------------------------------------------------------------------
# Entry point: full inputs in, full output out.
# ---------------------------------------------------------------------------

_CACHE = {}


def kernel(x, W, b, gamma, beta, edge_index):
    from concourse import bass_utils
    plan, in_maps = preprocess(x, W, gamma, beta, edge_index, n_cores=8)
    key = (plan.N, plan.D, plan.R_pairs, tuple(plan.calls))
    nc = _CACHE.get(key)
    if nc is None:
        nc = build_nc(plan)
        _CACHE[key] = nc
    res = None
    for attempt in range(3):
        try:
            res = bass_utils.run_bass_kernel_spmd(
                nc, in_maps, core_ids=list(range(plan.n_cores)))
            break
        except Exception:  # a wedged device usually recovers on retry
            if attempt == 2:
                raise
    outs = []
    for c, r in enumerate(res.results):
        o = r["outT"][:, :plan.SHARD].T            # [SHARD, D], slot order
        outs.append(o[np.argsort(plan.perm[c])])   # back to node order
    return np.ascontiguousarray(np.concatenate(outs, axis=0), dtype=np.float32)
